# revision 34
# baseline (speedup 1.0000x reference)
"""Fused multi-head-size-1 attention kernel for Trainium2 (Bass/Tile).

Problem: out = softmax((x_q Wq^T + bq)(x_k Wk^T + bk)^T / sqrt(D)) (x_v Wv^T + bv)
Shapes: B=8, QL=KL=2048, D=1024, fp32 in/out.

Sharding: data-parallel over batch. Core i processes batch i end-to-end;
no collectives. Host pre-transposes x/W to contraction-major layout and
casts matmul operands to bf16 (PE runs bf16 at 1 cycle/row vs 4 for fp32;
all accumulation stays fp32 in PSUM).

Per-core dataflow (everything resident in SBUF in bf16):
  phase 1: V[k',h] = xv @ Wv^T + bv (hc-major, xv staged through qt_sb's
           buffer; ones col appended for the softmax denominator),
           K^T[h,k'] = Wk @ xk^T (+bk), Q^T[h,q] = Wq @ xq^T (+bq)
  phase 2: per q-block: S^T[k',q] = K Q^T (PSUM, fp32), P^T = exp(S^T/32)
           (ScalarE, bf16 out), O[q,h] (+l) = P V_aug (PSUM, fp32),
           O = O * (1/l), DMA out (stores alternate HWDGE rings).

Schedule notes (from perfetto traces): each dma_start costs ~0.8us of
descriptor-gen on its sequencer and the first ~8us are fixed preamble, so
the DMA issue order is arranged so the only bytes ahead of the first
matmul are Wv's low half (ACT ring) and the first 256KB x chunk (SP
ring); everything else rides behind with multi-10us slack. PSUM drains
(bias adds, normalize muls) run on the otherwise-idle Vector engine.
"""

import numpy as np
import ml_dtypes

import concourse.bass as bass
import concourse.mybir as mybir
from concourse.bacc import Bacc
from concourse.tile import TileContext
from concourse.bass_utils import run_bass_kernel_spmd

B, QL, KL, D = 8, 2048, 2048, 1024
P = 128
NCORES = 8
DT = D // P          # 8 tiles along d/h
KT = KL // P         # 16 tiles along k'
XCH = 512            # x streaming chunk along s
QB = 512             # q block for the attention stage
F32 = mybir.dt.float32
BF16 = mybir.dt.bfloat16
SCALE = 1.0 / 32.0   # 1/sqrt(D)

# AV free-dim chunking over V's 1025 columns (1024 h + ones column for l).
# The l-carrying chunk goes first so the reciprocal overlaps the other
# chunks' matmuls.
AV_CHUNKS = [(684, 1025), (0, 342), (342, 684)]
AV_MAXW = 342


def build_bass() -> bass.Bass:
    # Bacc (not bare Bass): its finalize() runs the pass pipeline that splits
    # multi-semaphore waits into event semaphores (TRN2 allows 1 wait/inst).
    nc = Bacc()

    xqT = nc.declare_dram_parameter("xqT", [D, QL], BF16, isOutput=False)
    xkT = nc.declare_dram_parameter("xkT", [D, KL], BF16, isOutput=False)
    xvT = nc.declare_dram_parameter("xvT", [D, KL], BF16, isOutput=False)
    wqT = nc.declare_dram_parameter("wqT", [D, D], BF16, isOutput=False)
    wkT = nc.declare_dram_parameter("wkT", [D, D], BF16, isOutput=False)
    wvT = nc.declare_dram_parameter("wvT", [D, D], BF16, isOutput=False)
    bqp = nc.declare_dram_parameter("bqp", [P, DT], F32, isOutput=False)
    bkp = nc.declare_dram_parameter("bkp", [P, DT], F32, isOutput=False)
    bvt = nc.declare_dram_parameter("bvt", [P, D], F32, isOutput=False)
    out = nc.declare_dram_parameter("out", [QL, D], F32, isOutput=True)

    # contraction-major views: d = dt*128 + p
    xq_r = xqT[:].rearrange("(dt p) s -> p dt s", p=P)
    xk_r = xkT[:].rearrange("(dt p) s -> p dt s", p=P)
    xv_r = xvT[:].rearrange("(dt p) s -> p dt s", p=P)
    wq_r = wqT[:].rearrange("(dt p) h -> p dt h", p=P)
    wk_r = wkT[:].rearrange("(dt p) h -> p dt h", p=P)
    wv_r = wvT[:].rearrange("(dt p) h -> p dt h", p=P)

    with TileContext(nc) as tc:
        with (
            tc.tile_pool(name="persist", bufs=1) as persist,
            tc.tile_pool(name="consts", bufs=1) as consts,
        ):
            kt_sb = persist.tile([P, DT, KL], BF16, tag="kt")    # K^T[h%128, ht, k']
            v_sb = persist.tile([P, KT, D + 1], BF16, tag="v")   # V[k'%128, kt, h|1]
            qt_sb = persist.tile([P, DT, QL], BF16, tag="qt")    # Q^T[h%128, ht, q]

            bqp_sb = consts.tile([P, DT], F32, tag="bqp")
            bkp_sb = consts.tile([P, DT], F32, tag="bkp")
            bv_sb = consts.tile([P, D], F32, tag="bv")

            # ---------------- phase 1: projections ----------------
            with (
                tc.tile_pool(name="wpool", bufs=3) as wpool,
                tc.tile_pool(name="xpool", bufs=4) as xpool,
                tc.tile_pool(name="projp", bufs=6, space="PSUM") as projp,
                tc.tile_pool(name="warmp", bufs=1, space="PSUM") as warmp,
            ):
                # HAM warm-up: the PE sits idle ~7us waiting for the first
                # weight DMA, and would then pay ~3.4us of matmuls at the
                # cold 1.2GHz clock. Burn that idle window on dummy matmuls
                # over a zeroed scratch tile (dedicated PSUM bank, never
                # read) so the real stream starts at the warm 2.4GHz clock.
                scratch = consts.tile([P, 640], BF16, tag="scratch")
                nc.vector.memset(scratch[:], 0.0)
                wps = warmp.tile([P, 512], F32, tag="warm")
                for _ in range(31):
                    nc.tensor.matmul(
                        wps[:],
                        lhsT=scratch[:, 0:128],
                        rhs=scratch[:, 128:640],
                        start=True,
                        stop=True,
                    )
                # V first: its opening accumulation group only needs the low
                # 512-col half of Wv plus a small first x chunk, so the PE
                # starts ~2x sooner after the DMA preamble than K would
                # (K's first group needs all of Wk).
                # Ring schedule (rings are FIFO per issuing sequencer):
                #   ACT ring (scalar): Wv-hc0 Wv-hc1 | bkp bqp xk* | xq*
                #   SP  ring (sync):   xv0.. bv ..xv4 | Wk | Wq | out stores
                # Both Wv halves go on the ACT ring; all of xv streams on the
                # SP ring INTO qt_sb (Q^T's buffer — free until the Q phase,
                # and the exact same [P, DT, 2048] shape). The V loop runs
                # hc-major: the whole hc0 pass needs only Wv-hc0 (1MB), so
                # Wv-hc1 has ~40us of slack instead of being startup-critical.
                # V: out[s-tile, h-chunk] = sum_dt xvT[d,s-tile]^T @ WvT[d,h-chunk]
                # + bv (broadcast over rows), fused into the PSUM->SBUF move.
                w = wpool.tile([P, DT, D], BF16, tag="w")
                nc.scalar.dma_start(out=w[:, :, 0:512], in_=wv_r[:, :, 0:512])
                nc.scalar.dma_start(out=w[:, :, 512:1024], in_=wv_r[:, :, 512:1024])
                # bv on the ACT ring behind the weights, keeping the SP ring
                # pure x: the hc-major pass consumes x at ~150GB/s, right at
                # the early ring rate, so any foreign bytes there stall the PE
                nc.scalar.dma_start(out=bv_sb[:], in_=bvt[:])
                # fine-grained early chunks: on slow-DMA runs the hc0 pass
                # catches up with the x stream, and small leading chunks cap
                # the per-tile wait (sem granularity = one dma_start)
                v_chunks = [(0, 128), (128, 128), (256, 256), (512, 256),
                            (768, 256), (1024, 512), (1536, 512)]
                for c0, cw in v_chunks:
                    nc.sync.dma_start(out=qt_sb[:, :, c0:c0 + cw], in_=xv_r[:, :, c0:c0 + cw])
                # Hybrid group order: hc0-only for the first 4 s-tiles (so
                # only Wv-hc0 + 1MB of x gate the start), then st-major for
                # tiles 4-15 (each x tile serves BOTH output halves -> x
                # demand halves to ~73GB/s, robust to slow-DMA runs), then
                # hc1 for tiles 0-3 (x long resident).
                v_groups = (
                    [(st, 0) for st in range(4)]
                    + [(st, hc) for st in range(4, KL // P) for hc in (0, 1)]
                    + [(st, 1) for st in range(4)]
                )
                for st, hc in v_groups:
                    ps = projp.tile([P, 512], F32, tag="proj")
                    for dt in range(DT):
                        nc.tensor.matmul(
                            ps[:],
                            lhsT=qt_sb[:, dt, st * P:(st + 1) * P],
                            rhs=w[:, dt, hc * 512:(hc + 1) * 512],
                            start=(dt == 0),
                            stop=(dt == DT - 1),
                        )
                    nc.vector.tensor_add(
                        out=v_sb[:, st, hc * 512:(hc + 1) * 512],
                        in0=ps[:],
                        in1=bv_sb[:, hc * 512:(hc + 1) * 512],
                    )
                nc.vector.memset(v_sb[:, :, D], 1.0)  # ones column -> row sums

                # K^T: out[h-tile, k'-chunk] = sum_dt WkT[d,h-tile]^T @ xkT[d,k'-chunk]
                w = wpool.tile([P, DT, D], BF16, tag="w")
                nc.sync.dma_start(out=w[:], in_=wk_r)
                # biases ride the ACT ring behind the V x chunks; needed only
                # at bias-add time (~90us+)
                nc.scalar.dma_start(out=bkp_sb[:], in_=bkp[:])
                nc.scalar.dma_start(out=bqp_sb[:], in_=bqp[:])
                for cc in range(KL // XCH):
                    xc = xpool.tile([P, DT, XCH], BF16, tag="x")
                    nc.scalar.dma_start(out=xc[:], in_=xk_r[:, :, cc * XCH:(cc + 1) * XCH])
                    for ht in range(DT):
                        ps = projp.tile([P, XCH], F32, tag="proj")
                        for dt in range(DT):
                            nc.tensor.matmul(
                                ps[:],
                                lhsT=w[:, dt, ht * P:(ht + 1) * P],
                                rhs=xc[:, dt, :],
                                start=(dt == 0),
                                stop=(dt == DT - 1),
                            )
                        nc.vector.tensor_scalar_add(
                            out=kt_sb[:, ht, cc * XCH:(cc + 1) * XCH],
                            in0=ps[:],
                            scalar1=bkp_sb[:, ht:ht + 1],
                        )

                # Q^T: like K^T
                w = wpool.tile([P, DT, D], BF16, tag="w")
                nc.sync.dma_start(out=w[:], in_=wq_r)
                for cc in range(QL // XCH):
                    xc = xpool.tile([P, DT, XCH], BF16, tag="x")
                    nc.scalar.dma_start(out=xc[:], in_=xq_r[:, :, cc * XCH:(cc + 1) * XCH])
                    for ht in range(DT):
                        ps = projp.tile([P, XCH], F32, tag="proj")
                        for dt in range(DT):
                            nc.tensor.matmul(
                                ps[:],
                                lhsT=w[:, dt, ht * P:(ht + 1) * P],
                                rhs=xc[:, dt, :],
                                start=(dt == 0),
                                stop=(dt == DT - 1),
                            )
                        nc.vector.tensor_scalar_add(
                            out=qt_sb[:, ht, cc * XCH:(cc + 1) * XCH],
                            in0=ps[:],
                            scalar1=bqp_sb[:, ht:ht + 1],
                        )

            # ---------------- phase 2: attention ----------------
            with (
                tc.tile_pool(name="ptpool", bufs=3) as ptpool,
                tc.tile_pool(name="opool", bufs=4) as opool,
                tc.tile_pool(name="small", bufs=4) as small,
                # avp declared first so scorep lands on PSUM banks 6-7, which
                # the projection phase (projp, banks 0-5) never touched: the
                # first score matmul then has no write-after-read hazard
                # against the final projection drains
                tc.tile_pool(name="avp", bufs=6, space="PSUM") as avp,
                tc.tile_pool(name="scorep", bufs=2, space="PSUM") as scorep,
            ):
                for qb in range(QL // QB):
                    q0 = qb * QB
                    ptb = ptpool.tile([P, KT, QB], BF16, tag="pt")
                    # scores S^T[k', q], one k'-tile (one PSUM bank) per
                    # group; EXP drains each bank while the next fills
                    for kt in range(KT):
                        sp = scorep.tile([P, QB], F32, tag="score")
                        for ht in range(DT):
                            nc.tensor.matmul(
                                sp[:],
                                lhsT=kt_sb[:, ht, kt * P:(kt + 1) * P],
                                rhs=qt_sb[:, ht, q0:q0 + QB],
                                start=(ht == 0),
                                stop=(ht == DT - 1),
                            )
                        nc.scalar.activation(
                            out=ptb[:, kt, :],
                            in_=sp[:],
                            func=mybir.ActivationFunctionType.Exp,
                            scale=SCALE,
                        )
                    # AV + row sums + normalize, one q-tile (128 rows) at a time.
                    # kt outer / chunk inner: the stationary (P^T tile) is
                    # reused across the 3 V chunks -> 1/3 the LDWEIGHTS.
                    for qt4 in range(QB // P):
                        qrow = q0 + qt4 * P
                        rl = small.tile([P, 1], F32, tag="rl")
                        ob = opool.tile([P, D], F32, tag="o")
                        for ci, (h0, h1) in enumerate(AV_CHUNKS):
                            av = avp.tile([P, AV_MAXW], F32, tag="av")
                            for kt in range(KT):
                                nc.tensor.matmul(
                                    av[:, :h1 - h0],
                                    lhsT=ptb[:, kt, qt4 * P:(qt4 + 1) * P],
                                    rhs=v_sb[:, kt, h0:h1],
                                    start=(kt == 0),
                                    stop=(kt == KT - 1),
                                )
                            if ci == 0:
                                # l (row sums) is the last column (global idx D)
                                nc.vector.reciprocal(rl[:], av[:, D - h0:D - h0 + 1])
                            w_ = min(h1, D) - h0
                            last_tile = qb == QL // QB - 1 and qt4 == QB // P - 1
                            if last_tile and ci == len(AV_CHUNKS) - 1:
                                # final chunk of the whole kernel: normalize
                                # and store in two halves so the first store's
                                # descriptor-gen overlaps the second normalize
                                half = w_ // 2
                                for j, (a, b) in enumerate([(0, half), (half, w_)]):
                                    nc.vector.tensor_scalar_mul(
                                        out=ob[:, h0 + a:h0 + b],
                                        in0=av[:, a:b],
                                        scalar1=rl[:],
                                    )
                                    eng = nc.sync if j == 0 else nc.scalar
                                    eng.dma_start(
                                        out=out[qrow:qrow + P, h0 + a:h0 + b],
                                        in_=ob[:, h0 + a:h0 + b],
                                    )
                                continue
                            nc.vector.tensor_scalar_mul(
                                out=ob[:, h0:h0 + w_],
                                in0=av[:, :w_],
                                scalar1=rl[:],
                            )
                            if last_tile:
                                # very last q-tile: stream the output per chunk
                                # across BOTH rings so the final DMA isn't
                                # serialized behind all three normalizes
                                eng = nc.sync if ci % 2 == 0 else nc.scalar
                                eng.dma_start(
                                    out=out[qrow:qrow + P, h0:h0 + w_],
                                    in_=ob[:, h0:h0 + w_],
                                )
                        if not (qb == QL // QB - 1 and qt4 == QB // P - 1):
                            # alternate stores across the two HWDGE rings
                            eng = nc.sync if (qb * (QB // P) + qt4) % 2 == 0 else nc.scalar
                            eng.dma_start(out=out[qrow:qrow + P, :], in_=ob[:])

    nc.finalize()
    return nc


def prepare_in_maps(q_embd, k_embd, v_embd, Wq, bq, Wk, bk, Wv, bv):
    bf16 = ml_dtypes.bfloat16
    f32 = np.float32

    def t_cast(x):  # [B, L, D] -> [B, D, L] bf16
        return np.ascontiguousarray(np.swapaxes(np.asarray(x, f32), 1, 2)).astype(bf16)

    xqT = t_cast(q_embd)
    xkT = t_cast(k_embd)
    xvT = t_cast(v_embd)
    wqT = np.ascontiguousarray(np.asarray(Wq, f32).T).astype(bf16)
    wkT = np.ascontiguousarray(np.asarray(Wk, f32).T).astype(bf16)
    wvT = np.ascontiguousarray(np.asarray(Wv, f32).T).astype(bf16)
    bqp = np.ascontiguousarray(np.asarray(bq, f32).reshape(DT, P).T)
    bkp = np.ascontiguousarray(np.asarray(bk, f32).reshape(DT, P).T)
    bvt = np.ascontiguousarray(np.tile(np.asarray(bv, f32)[None, :], (P, 1)))

    return [
        {
            "xqT": xqT[i], "xkT": xkT[i], "xvT": xvT[i],
            "wqT": wqT, "wkT": wkT, "wvT": wvT,
            "bqp": bqp, "bkp": bkp, "bvt": bvt,
        }
        for i in range(NCORES)
    ]


_NC_CACHE = None


def get_nc() -> bass.Bass:
    global _NC_CACHE
    if _NC_CACHE is None:
        _NC_CACHE = build_bass()
    return _NC_CACHE


def run_on_device(in_maps, trace=False, **kwargs):
    return run_bass_kernel_spmd(get_nc(), in_maps, list(range(NCORES)), trace=trace, **kwargs)


def kernel(q_embd, k_embd, v_embd, Wq, bq, Wk, bk, Wv, bv):
    in_maps = prepare_in_maps(q_embd, k_embd, v_embd, Wq, bq, Wk, bk, Wv, bv)
    res = run_on_device(in_maps)
    return np.stack([r["out"] for r in res.results], axis=0)



# revision 35
# speedup vs baseline: 1.0013x; 1.0013x over previous
"""Fused multi-head-size-1 attention kernel for Trainium2 (Bass/Tile).

Problem: out = softmax((x_q Wq^T + bq)(x_k Wk^T + bk)^T / sqrt(D)) (x_v Wv^T + bv)
Shapes: B=8, QL=KL=2048, D=1024, fp32 in/out.

Sharding: data-parallel over batch. Core i processes batch i end-to-end;
no collectives. Host pre-transposes x/W to contraction-major layout and
casts matmul operands to bf16 (PE runs bf16 at 1 cycle/row vs 4 for fp32;
all accumulation stays fp32 in PSUM).

Per-core dataflow (everything resident in SBUF in bf16):
  phase 1: V[k',h] = xv @ Wv^T + bv (hc-major, xv staged through qt_sb's
           buffer; ones col appended for the softmax denominator),
           K^T[h,k'] = Wk @ xk^T (+bk), Q^T[h,q] = Wq @ xq^T (+bq)
  phase 2: per q-block: S^T[k',q] = K Q^T (PSUM, fp32), P^T = exp(S^T/32)
           (ScalarE, bf16 out), O[q,h] (+l) = P V_aug (PSUM, fp32),
           O = O * (1/l), DMA out (stores alternate HWDGE rings).

Schedule notes (from perfetto traces): each dma_start costs ~0.8us of
descriptor-gen on its sequencer and the first ~8us are fixed preamble, so
the DMA issue order is arranged so the only bytes ahead of the first
matmul are Wv's low half (ACT ring) and the first 256KB x chunk (SP
ring); everything else rides behind with multi-10us slack. PSUM drains
(bias adds, normalize muls) run on the otherwise-idle Vector engine.
"""

import numpy as np
import ml_dtypes

import concourse.bass as bass
import concourse.mybir as mybir
from concourse.bacc import Bacc
from concourse.tile import TileContext
from concourse.bass_utils import run_bass_kernel_spmd

B, QL, KL, D = 8, 2048, 2048, 1024
P = 128
NCORES = 8
DT = D // P          # 8 tiles along d/h
KT = KL // P         # 16 tiles along k'
XCH = 512            # x streaming chunk along s
QB = 512             # q block for the attention stage
F32 = mybir.dt.float32
BF16 = mybir.dt.bfloat16
SCALE = 1.0 / 32.0   # 1/sqrt(D)

# AV free-dim chunking over V's 1025 columns (1024 h + ones column for l).
# The l-carrying chunk goes first so the reciprocal overlaps the other
# chunks' matmuls.
AV_CHUNKS = [(684, 1025), (0, 342), (342, 684)]
AV_MAXW = 342


def build_bass() -> bass.Bass:
    # Bacc (not bare Bass): its finalize() runs the pass pipeline that splits
    # multi-semaphore waits into event semaphores (TRN2 allows 1 wait/inst).
    nc = Bacc()

    xqT = nc.declare_dram_parameter("xqT", [D, QL], BF16, isOutput=False)
    xkT = nc.declare_dram_parameter("xkT", [D, KL], BF16, isOutput=False)
    xvT = nc.declare_dram_parameter("xvT", [D, KL], BF16, isOutput=False)
    wqT = nc.declare_dram_parameter("wqT", [D, D], BF16, isOutput=False)
    wkT = nc.declare_dram_parameter("wkT", [D, D], BF16, isOutput=False)
    wvT = nc.declare_dram_parameter("wvT", [D, D], BF16, isOutput=False)
    bqp = nc.declare_dram_parameter("bqp", [P, DT], F32, isOutput=False)
    bkp = nc.declare_dram_parameter("bkp", [P, DT], F32, isOutput=False)
    bvt = nc.declare_dram_parameter("bvt", [P, D], F32, isOutput=False)
    out = nc.declare_dram_parameter("out", [QL, D], F32, isOutput=True)

    # contraction-major views: d = dt*128 + p
    xq_r = xqT[:].rearrange("(dt p) s -> p dt s", p=P)
    xk_r = xkT[:].rearrange("(dt p) s -> p dt s", p=P)
    xv_r = xvT[:].rearrange("(dt p) s -> p dt s", p=P)
    wq_r = wqT[:].rearrange("(dt p) h -> p dt h", p=P)
    wk_r = wkT[:].rearrange("(dt p) h -> p dt h", p=P)
    wv_r = wvT[:].rearrange("(dt p) h -> p dt h", p=P)

    with TileContext(nc) as tc:
        with (
            tc.tile_pool(name="persist", bufs=1) as persist,
            tc.tile_pool(name="consts", bufs=1) as consts,
        ):
            kt_sb = persist.tile([P, DT, KL], BF16, tag="kt")    # K^T[h%128, ht, k']
            v_sb = persist.tile([P, KT, D + 1], BF16, tag="v")   # V[k'%128, kt, h|1]
            qt_sb = persist.tile([P, DT, QL], BF16, tag="qt")    # Q^T[h%128, ht, q]

            bqp_sb = consts.tile([P, DT], F32, tag="bqp")
            bkp_sb = consts.tile([P, DT], F32, tag="bkp")
            bv_sb = consts.tile([P, D], F32, tag="bv")

            # ---------------- phase 1: projections ----------------
            with (
                tc.tile_pool(name="wpool", bufs=3) as wpool,
                tc.tile_pool(name="xpool", bufs=4) as xpool,
                tc.tile_pool(name="projp", bufs=6, space="PSUM") as projp,
                tc.tile_pool(name="warmp", bufs=1, space="PSUM") as warmp,
            ):
                # HAM warm-up: the PE sits idle ~7us waiting for the first
                # weight DMA, and would then pay ~3.4us of matmuls at the
                # cold 1.2GHz clock. Burn that idle window on dummy matmuls
                # over a zeroed scratch tile (dedicated PSUM bank, never
                # read) so the real stream starts at the warm 2.4GHz clock.
                scratch = consts.tile([P, 640], BF16, tag="scratch")
                nc.vector.memset(scratch[:], 0.0)
                wps = warmp.tile([P, 512], F32, tag="warm")
                for _ in range(31):
                    nc.tensor.matmul(
                        wps[:],
                        lhsT=scratch[:, 0:128],
                        rhs=scratch[:, 128:640],
                        start=True,
                        stop=True,
                    )
                # V first: its opening accumulation group only needs the low
                # 512-col half of Wv plus a small first x chunk, so the PE
                # starts ~2x sooner after the DMA preamble than K would
                # (K's first group needs all of Wk).
                # Ring schedule (rings are FIFO per issuing sequencer):
                #   ACT ring (scalar): Wv-hc0 Wv-hc1 | bkp bqp xk* | xq*
                #   SP  ring (sync):   xv0.. bv ..xv4 | Wk | Wq | out stores
                # Both Wv halves go on the ACT ring; all of xv streams on the
                # SP ring INTO qt_sb (Q^T's buffer — free until the Q phase,
                # and the exact same [P, DT, 2048] shape). The V loop runs
                # hc-major: the whole hc0 pass needs only Wv-hc0 (1MB), so
                # Wv-hc1 has ~40us of slack instead of being startup-critical.
                # V: out[s-tile, h-chunk] = sum_dt xvT[d,s-tile]^T @ WvT[d,h-chunk]
                # + bv (broadcast over rows), fused into the PSUM->SBUF move.
                w = wpool.tile([P, DT, D], BF16, tag="w")
                nc.scalar.dma_start(out=w[:, :, 0:512], in_=wv_r[:, :, 0:512])
                nc.scalar.dma_start(out=w[:, :, 512:1024], in_=wv_r[:, :, 512:1024])
                # bv on the ACT ring behind the weights, keeping the SP ring
                # pure x: the hc-major pass consumes x at ~150GB/s, right at
                # the early ring rate, so any foreign bytes there stall the PE
                nc.scalar.dma_start(out=bv_sb[:], in_=bvt[:])
                # fine-grained early chunks: on slow-DMA runs the hc0 pass
                # catches up with the x stream, and small leading chunks cap
                # the per-tile wait (sem granularity = one dma_start)
                v_chunks = [(0, 128), (128, 128), (256, 128), (384, 128),
                            (512, 256), (768, 256), (1024, 512), (1536, 512)]
                for c0, cw in v_chunks:
                    nc.sync.dma_start(out=qt_sb[:, :, c0:c0 + cw], in_=xv_r[:, :, c0:c0 + cw])
                # Hybrid group order: hc0-only for the first 4 s-tiles (so
                # only Wv-hc0 + 1MB of x gate the start), then st-major for
                # tiles 4-15 (each x tile serves BOTH output halves -> x
                # demand halves to ~73GB/s, robust to slow-DMA runs), then
                # hc1 for tiles 0-3 (x long resident).
                v_groups = (
                    [(st, 0) for st in range(4)]
                    + [(st, hc) for st in range(4, KL // P) for hc in (0, 1)]
                    + [(st, 1) for st in range(4)]
                )
                for st, hc in v_groups:
                    ps = projp.tile([P, 512], F32, tag="proj")
                    for dt in range(DT):
                        nc.tensor.matmul(
                            ps[:],
                            lhsT=qt_sb[:, dt, st * P:(st + 1) * P],
                            rhs=w[:, dt, hc * 512:(hc + 1) * 512],
                            start=(dt == 0),
                            stop=(dt == DT - 1),
                        )
                    nc.vector.tensor_add(
                        out=v_sb[:, st, hc * 512:(hc + 1) * 512],
                        in0=ps[:],
                        in1=bv_sb[:, hc * 512:(hc + 1) * 512],
                    )
                nc.vector.memset(v_sb[:, :, D], 1.0)  # ones column -> row sums

                # K^T: out[h-tile, k'-chunk] = sum_dt WkT[d,h-tile]^T @ xkT[d,k'-chunk]
                w = wpool.tile([P, DT, D], BF16, tag="w")
                nc.sync.dma_start(out=w[:], in_=wk_r)
                # biases ride the ACT ring behind the V x chunks; needed only
                # at bias-add time (~90us+)
                nc.scalar.dma_start(out=bkp_sb[:], in_=bkp[:])
                nc.scalar.dma_start(out=bqp_sb[:], in_=bqp[:])
                for cc in range(KL // XCH):
                    xc = xpool.tile([P, DT, XCH], BF16, tag="x")
                    nc.scalar.dma_start(out=xc[:], in_=xk_r[:, :, cc * XCH:(cc + 1) * XCH])
                    for ht in range(DT):
                        ps = projp.tile([P, XCH], F32, tag="proj")
                        for dt in range(DT):
                            nc.tensor.matmul(
                                ps[:],
                                lhsT=w[:, dt, ht * P:(ht + 1) * P],
                                rhs=xc[:, dt, :],
                                start=(dt == 0),
                                stop=(dt == DT - 1),
                            )
                        nc.vector.tensor_scalar_add(
                            out=kt_sb[:, ht, cc * XCH:(cc + 1) * XCH],
                            in0=ps[:],
                            scalar1=bkp_sb[:, ht:ht + 1],
                        )

                # Q^T: like K^T
                w = wpool.tile([P, DT, D], BF16, tag="w")
                nc.sync.dma_start(out=w[:], in_=wq_r)
                for cc in range(QL // XCH):
                    xc = xpool.tile([P, DT, XCH], BF16, tag="x")
                    nc.scalar.dma_start(out=xc[:], in_=xq_r[:, :, cc * XCH:(cc + 1) * XCH])
                    for ht in range(DT):
                        ps = projp.tile([P, XCH], F32, tag="proj")
                        for dt in range(DT):
                            nc.tensor.matmul(
                                ps[:],
                                lhsT=w[:, dt, ht * P:(ht + 1) * P],
                                rhs=xc[:, dt, :],
                                start=(dt == 0),
                                stop=(dt == DT - 1),
                            )
                        nc.vector.tensor_scalar_add(
                            out=qt_sb[:, ht, cc * XCH:(cc + 1) * XCH],
                            in0=ps[:],
                            scalar1=bqp_sb[:, ht:ht + 1],
                        )

            # ---------------- phase 2: attention ----------------
            with (
                tc.tile_pool(name="ptpool", bufs=3) as ptpool,
                tc.tile_pool(name="opool", bufs=4) as opool,
                tc.tile_pool(name="small", bufs=4) as small,
                # avp declared first so scorep lands on PSUM banks 6-7, which
                # the projection phase (projp, banks 0-5) never touched: the
                # first score matmul then has no write-after-read hazard
                # against the final projection drains
                tc.tile_pool(name="avp", bufs=6, space="PSUM") as avp,
                tc.tile_pool(name="scorep", bufs=2, space="PSUM") as scorep,
            ):
                for qb in range(QL // QB):
                    q0 = qb * QB
                    ptb = ptpool.tile([P, KT, QB], BF16, tag="pt")
                    # scores S^T[k', q], one k'-tile (one PSUM bank) per
                    # group; EXP drains each bank while the next fills
                    for kt in range(KT):
                        sp = scorep.tile([P, QB], F32, tag="score")
                        for ht in range(DT):
                            nc.tensor.matmul(
                                sp[:],
                                lhsT=kt_sb[:, ht, kt * P:(kt + 1) * P],
                                rhs=qt_sb[:, ht, q0:q0 + QB],
                                start=(ht == 0),
                                stop=(ht == DT - 1),
                            )
                        nc.scalar.activation(
                            out=ptb[:, kt, :],
                            in_=sp[:],
                            func=mybir.ActivationFunctionType.Exp,
                            scale=SCALE,
                        )
                    # AV + row sums + normalize, one q-tile (128 rows) at a time.
                    # kt outer / chunk inner: the stationary (P^T tile) is
                    # reused across the 3 V chunks -> 1/3 the LDWEIGHTS.
                    for qt4 in range(QB // P):
                        qrow = q0 + qt4 * P
                        rl = small.tile([P, 1], F32, tag="rl")
                        ob = opool.tile([P, D], F32, tag="o")
                        for ci, (h0, h1) in enumerate(AV_CHUNKS):
                            av = avp.tile([P, AV_MAXW], F32, tag="av")
                            for kt in range(KT):
                                nc.tensor.matmul(
                                    av[:, :h1 - h0],
                                    lhsT=ptb[:, kt, qt4 * P:(qt4 + 1) * P],
                                    rhs=v_sb[:, kt, h0:h1],
                                    start=(kt == 0),
                                    stop=(kt == KT - 1),
                                )
                            if ci == 0:
                                # l (row sums) is the last column (global idx D)
                                nc.vector.reciprocal(rl[:], av[:, D - h0:D - h0 + 1])
                            w_ = min(h1, D) - h0
                            last_tile = qb == QL // QB - 1 and qt4 == QB // P - 1
                            if last_tile and ci == len(AV_CHUNKS) - 1:
                                # final chunk of the whole kernel: normalize
                                # and store in two halves so the first store's
                                # descriptor-gen overlaps the second normalize
                                half = w_ // 2
                                for j, (a, b) in enumerate([(0, half), (half, w_)]):
                                    nc.vector.tensor_scalar_mul(
                                        out=ob[:, h0 + a:h0 + b],
                                        in0=av[:, a:b],
                                        scalar1=rl[:],
                                    )
                                    eng = nc.sync if j == 0 else nc.scalar
                                    eng.dma_start(
                                        out=out[qrow:qrow + P, h0 + a:h0 + b],
                                        in_=ob[:, h0 + a:h0 + b],
                                    )
                                continue
                            nc.vector.tensor_scalar_mul(
                                out=ob[:, h0:h0 + w_],
                                in0=av[:, :w_],
                                scalar1=rl[:],
                            )
                            if last_tile:
                                # very last q-tile: stream the output per chunk
                                # across BOTH rings so the final DMA isn't
                                # serialized behind all three normalizes
                                eng = nc.sync if ci % 2 == 0 else nc.scalar
                                eng.dma_start(
                                    out=out[qrow:qrow + P, h0:h0 + w_],
                                    in_=ob[:, h0:h0 + w_],
                                )
                        if not (qb == QL // QB - 1 and qt4 == QB // P - 1):
                            # alternate stores across the two HWDGE rings
                            eng = nc.sync if (qb * (QB // P) + qt4) % 2 == 0 else nc.scalar
                            eng.dma_start(out=out[qrow:qrow + P, :], in_=ob[:])

    nc.finalize()
    return nc


def prepare_in_maps(q_embd, k_embd, v_embd, Wq, bq, Wk, bk, Wv, bv):
    bf16 = ml_dtypes.bfloat16
    f32 = np.float32

    def t_cast(x):  # [B, L, D] -> [B, D, L] bf16
        return np.ascontiguousarray(np.swapaxes(np.asarray(x, f32), 1, 2)).astype(bf16)

    xqT = t_cast(q_embd)
    xkT = t_cast(k_embd)
    xvT = t_cast(v_embd)
    wqT = np.ascontiguousarray(np.asarray(Wq, f32).T).astype(bf16)
    wkT = np.ascontiguousarray(np.asarray(Wk, f32).T).astype(bf16)
    wvT = np.ascontiguousarray(np.asarray(Wv, f32).T).astype(bf16)
    bqp = np.ascontiguousarray(np.asarray(bq, f32).reshape(DT, P).T)
    bkp = np.ascontiguousarray(np.asarray(bk, f32).reshape(DT, P).T)
    bvt = np.ascontiguousarray(np.tile(np.asarray(bv, f32)[None, :], (P, 1)))

    return [
        {
            "xqT": xqT[i], "xkT": xkT[i], "xvT": xvT[i],
            "wqT": wqT, "wkT": wkT, "wvT": wvT,
            "bqp": bqp, "bkp": bkp, "bvt": bvt,
        }
        for i in range(NCORES)
    ]


_NC_CACHE = None


def get_nc() -> bass.Bass:
    global _NC_CACHE
    if _NC_CACHE is None:
        _NC_CACHE = build_bass()
    return _NC_CACHE


def run_on_device(in_maps, trace=False, **kwargs):
    return run_bass_kernel_spmd(get_nc(), in_maps, list(range(NCORES)), trace=trace, **kwargs)


def kernel(q_embd, k_embd, v_embd, Wq, bq, Wk, bk, Wv, bv):
    in_maps = prepare_in_maps(q_embd, k_embd, v_embd, Wq, bq, Wk, bk, Wv, bv)
    res = run_on_device(in_maps)
    return np.stack([r["out"] for r in res.results], axis=0)

